# revision 41
# baseline (speedup 1.0000x reference)
"""Trainium2 Bass kernel for nn_BondDecoder (topk_masking).

Strategy:
  - Data-parallel over batch: 64 batches -> 8 cores x 8 slots, compacted to
    unmasked positions per batch (sorted + dealt so SPMD slot sizes match).
  - All PE inputs in f16 (f32r pays 4x cyc/row when N < 256; f16 is 1 always).
  - The static map A = C_src - C_tgt*(1-t_i)(1-t_j) is precomputed on the
    HOST and DMA'd in (f16): one add replaces the old iota/is_equal chain.
  - VIRTUAL PAD COLUMNS instead of -100 mask matmuls: each slot gets 8 extra
    zero-x columns, so every padded column of head h has the identical value
    e_i = exp(q_i . bk_h).  S and the loss row sum are corrected with tiny
    [il,8]/[il,1] ops using the last (guaranteed-pad) column:
        S_true = S_raw - npad * E[:, w-1],  R_true = R_raw - npad * msq[:, w-1]
  - Per chunk: 8 head-score matmuls into two 4-bank PSUM tiles; two batched
    exp4 activations; per-head row sums S_h via tensor_scalar+accum_out (DVE
    4x path); r = +/-valid/S (sign folds inc-dec); u = sum_h r_h E_h via
    8 ts_mul + Pool add tree; loss rows R/T via Pool squares + DVE ts-accum.
  - Projection biases are folded into the PSUM->SBUF copies (scalar.add /
    tensor_scalar add) instead of K=1 matmuls.
  - Leftover rows (256..n) of big slots are CONSOLIDATED into one shared
    final chunk (per-row A/t_j/valid data host-packed into a virtual slot).
  - Final: ones-vector matmul over partitions + 3-column fold per slot.
"""

import sys
from contextlib import ExitStack

if "/opt/trn_rl_repo" not in sys.path:
    sys.path.insert(0, "/opt/trn_rl_repo")

import numpy as np

import concourse.bacc as bacc
import concourse.tile as tile
from concourse import bass_utils, mybir

L, B, DIM = 512, 64, 256
H, HD, MB = 4, 64, 6
NCORES = 8
BPC = B // NCORES  # slots per core

F32 = mybir.dt.float32
F16 = mybir.dt.float16
EDT = mybir.dt.float16
NP_EDT = np.float16

_CACHE = {}


def _cons_split(slot_n):
    """Slots whose rows 256..n go to the consolidated chunk. PE matmul
    output base partitions must be 32-aligned ({0,32,64}), so each slot
    gets a 32-wide lane (up to 3 slots, leftover count <= 32 each)."""
    bigs, offs = [], {}
    for s, n in enumerate(slot_n):
        c = n - 256
        if 0 < c <= 32 and len(bigs) < 3:
            offs[s] = 32 * len(bigs)
            bigs.append(s)
    tot = 0
    if bigs:
        last = bigs[-1]
        tot = offs[last] + (slot_n[last] - 256)
    return bigs, offs, tot


def _build_program(slot_n, nmax, totc):
    nc = bacc.Bacc(
        "TRN2",
        target_bir_lowering=False,
        debug=False,
        enable_asserts=False,
        num_devices=NCORES,
    )
    AL = mybir.AluOpType
    AF = mybir.ActivationFunctionType

    bigs, cons_off, ptot = _cons_split(slot_n)
    wmax = nmax + 8

    xT_d = nc.dram_tensor("xT", [DIM, totc], F16, kind="ExternalInput").ap()
    wall_d = nc.dram_tensor("wall", [4, DIM, DIM], F16, kind="ExternalInput").ap()
    wbias_d = nc.dram_tensor("wbias", [4, 128, 2], F32, kind="ExternalInput").ap()
    amap_d = nc.dram_tensor(
        "amap", [BPC + 1, 512, wmax + 16], F16, kind="ExternalInput"
    ).ap()
    tjrow_d = nc.dram_tensor("tjrow", [BPC, wmax], F16, kind="ExternalInput").ap()
    loss_d = nc.dram_tensor("loss", [1, BPC], F32, kind="ExternalOutput").ap()

    with ExitStack() as ctx:
        tc = ctx.enter_context(tile.TileContext(nc))
        singles = ctx.enter_context(tc.tile_pool(name="singles", bufs=1))
        xapool = ctx.enter_context(tc.tile_pool(name="xapool", bufs=5))
        xpool = ctx.enter_context(tc.tile_pool(name="xpool", bufs=BPC + 1))
        qk = ctx.enter_context(tc.tile_pool(name="qk", bufs=BPC))
        epool = ctx.enter_context(tc.tile_pool(name="epool", bufs=2))
        work = ctx.enter_context(tc.tile_pool(name="work", bufs=2))
        small = ctx.enter_context(tc.tile_pool(name="small", bufs=8))
        pscp = ctx.enter_context(tc.tile_pool(name="psc", bufs=2, space="PSUM"))

        queues = [nc.sync, nc.gpsimd]

        # ---- weights / constants (once per core) ----
        w0, w1, wbias = [], [], []
        for p in range(4):
            t0 = singles.tile([128, DIM], F16, tag=f"w0_{p}", name=f"w0_{p}")
            nc.gpsimd.dma_start(out=t0, in_=wall_d[p, 0:128, :])
            t1 = singles.tile([128, DIM], F16, tag=f"w1_{p}", name=f"w1_{p}")
            nc.gpsimd.dma_start(out=t1, in_=wall_d[p, 128:256, :])
            tb = singles.tile([128, 2], F32, tag=f"wbias_{p}", name=f"wbias_{p}")
            nc.gpsimd.dma_start(out=tb, in_=wbias_d[p])
            w0.append(t0)
            w1.append(t1)
            wbias.append(tb)

        ones128 = singles.tile([128, 1], F32, tag="ones128")
        nc.vector.memset(ones128, 1.0)
        warm = singles.tile([1, 8], F32, tag="warm")
        nc.vector.memset(warm, 0.0)
        nc.scalar.activation(out=warm, in_=warm, func=AF.Exp)
        ones1 = singles.tile([1, 128], F16, tag="ones1")
        nc.vector.memset(ones1, 1.0)
        res = singles.tile([128, BPC * 3], F32, tag="res")
        nc.vector.memset(res, 0.0)

        # per-slot persistent handles (for the consolidated tail chunk)
        S_mrow, S_qkt, S_xb = {}, {}, {}
        holder = {}

        def load_slot(s, n, off):
            w = n + 8
            xall = xapool.tile([128, 2, wmax], F16, tag="xall")
            nc.sync.dma_start(
                out=xall[:, :, 0:w],
                in_=xT_d[0:256, off : off + w].rearrange("(a p) c -> p a c", p=128),
            )
            amtile = xpool.tile([128, 4, wmax + 16], F16, tag="am", name=f"am{s}")
            nc.sync.dma_start(
                out=amtile,
                in_=amap_d[s].rearrange("(c p) w -> p c w", p=128),
            )
            S_mrow[s] = amtile[0:1, 3, 0:wmax]
            tjr = xpool.tile([1, wmax], F16, tag="tjr", name=f"tjr{s}")
            nc.sync.dma_start(out=tjr, in_=tjrow_d[s : s + 1, :])
            tbc = xpool.tile([128, wmax], F16, tag="tbc", name=f"tbc{s}")
            nc.gpsimd.partition_broadcast(tbc[:, 0:w], tjr[0:1, 0:w])
            return xall, tbc, amtile

        def project(s, n, xall):
            w = n + 8
            qkt = {}
            for br in range(2):
                pp = pscp.tile([128, 4, 512], F32, tag="psc", name=f"pp{br}")
                for g in range(2):
                    for j, p in enumerate((2 * br, 2 * br + 1)):
                        mg = slice(128 * g, 128 * g + 128)
                        sl = 2 * g + j
                        nc.tensor.matmul(
                            pp[:, sl, 0:w], w0[p][:, mg], xall[:, 0, 0:w],
                            start=True, stop=False,
                        )
                        nc.tensor.matmul(
                            pp[:, sl, 0:w], w1[p][:, mg], xall[:, 1, 0:w],
                            start=False, stop=True,
                        )
                for g in range(2):
                    qt = qk.tile([128, 2, wmax], F16, tag=f"qk{br}{g}",
                                 name=f"qk{s}{br}{g}")
                    for j in range(2):
                        p = 2 * br + j
                        bias = wbias[p][:, g : g + 1]
                        srcap = pp[:, 2 * g + j, 0:w]
                        dst = qt[:, j, 0:w]
                        if j == 0:
                            nc.scalar.add(dst, srcap, bias)
                        else:
                            nc.vector.tensor_scalar(
                                out=dst, in0=srcap, scalar1=bias,
                                scalar2=None, op0=AL.add)
                    qkt[(br, g)] = qt
            S_qkt[s] = qkt
            return qkt

        def chunk_tail(il, w, amap_c, tw, vsrow, rescols):
            """Post-exp elementwise chain. vsrow: [il, 16] f16 slice with
            cols 0:8=+/-valid, 8=-t_i, 9=npad, 10=-npad.
            rescols: [(p0, p1, col), ...]."""
            E = holder["E"]
            Sall = small.tile([128, 8], F32, tag="Sall")
            sdum = work.tile([128, wmax], F16, tag="sdum")
            for h8 in range(8):
                nc.vector.tensor_scalar(
                    out=sdum[0:il, 0:w], in0=E[0:il, h8, 0:w],
                    scalar1=1.0, scalar2=0.0, op0=AL.mult, op1=AL.add,
                    accum_out=Sall[0:il, h8 : h8 + 1],
                )
            # S_true = S_raw - npad * E[:, w-1] (identical pad columns)
            Sc = small.tile([128, 8], F32, tag="Sc")
            nc.vector.scalar_tensor_tensor(
                out=Sc[0:il], in0=E[0:il, 0:8, w - 1],
                scalar=vsrow[:, 10:11], in1=Sall[0:il],
                op0=AL.mult, op1=AL.add,
            )
            r = small.tile([128, 8], F32, tag="r")
            nc.vector.reciprocal(out=r[0:il], in_=Sc[0:il])
            nc.vector.tensor_tensor(
                out=r[0:il], in0=r[0:il], in1=vsrow[:, 0:8], op=AL.mult)

            scr8 = work.tile([128, 8, wmax], F16, tag="scr8")
            for h8 in range(8):
                eng = nc.gpsimd if h8 >= 6 else nc.vector
                eng.tensor_scalar_mul(
                    out=scr8[0:il, h8, 0:w], in0=E[0:il, h8, 0:w],
                    scalar1=r[0:il, h8 : h8 + 1],
                )
            v4 = work.tile([128, 4, wmax], F16, tag="v4")
            nc.gpsimd.tensor_add(
                v4[0:il, :, 0:w], scr8[0:il, 0:4, 0:w], scr8[0:il, 4:8, 0:w])
            v2 = work.tile([128, 2, wmax], F16, tag="v2")
            nc.gpsimd.tensor_add(
                v2[0:il, :, 0:w], v4[0:il, 0:2, 0:w], v4[0:il, 2:4, 0:w])
            u1 = work.tile([128, wmax], F16, tag="u1")
            nc.gpsimd.tensor_add(
                u1[0:il, 0:w], v2[0:il, 0, 0:w], v2[0:il, 1, 0:w])
            mA = work.tile([128, wmax], F16, tag="mA")
            nc.gpsimd.tensor_add(mA[0:il, 0:w], u1[0:il, 0:w], amap_c)
            msq = work.tile([128, wmax], F16, tag="msq")
            msqt = work.tile([128, wmax], F16, tag="msqt")
            Racc = small.tile([128, 1], F32, tag="Racc")
            Rc = small.tile([128, 1], F32, tag="Rc")
            Tacc = small.tile([128, 1], F32, tag="Tacc")
            nc.gpsimd.tensor_mul(msq[0:il, 0:w], mA[0:il, 0:w], mA[0:il, 0:w])
            nc.gpsimd.tensor_mul(msqt[0:il, 0:w], msq[0:il, 0:w], tw)
            nc.vector.tensor_scalar(
                out=sdum[0:il, 0:w], in0=msq[0:il, 0:w],
                scalar1=1.0, scalar2=0.0, op0=AL.mult, op1=AL.add,
                accum_out=Racc[0:il],
            )
            nc.vector.tensor_scalar(
                out=sdum[0:il, 0:w], in0=msqt[0:il, 0:w],
                scalar1=1.0, scalar2=0.0, op0=AL.mult, op1=AL.add,
                accum_out=Tacc[0:il],
            )
            # R_true = R_raw - npad * msq[:, w-1]
            nc.vector.scalar_tensor_tensor(
                out=Rc[0:il],
                in0=msq[0:il, w - 1 : w],
                scalar=vsrow[:, 10:11],
                in1=Racc[0:il],
                op0=AL.mult, op1=AL.add,
            )
            for p0, p1, col in rescols:
                nc.vector.scalar_tensor_tensor(
                    out=res[p0:p1, col : col + 1],
                    in0=Tacc[p0:p1],
                    scalar=vsrow[p0:p1, 8:9],
                    in1=Rc[p0:p1],
                    op0=AL.mult, op1=AL.add,
                )

        # ---- per-slot processing (chunks 0..1 only; rest consolidated) ----
        def do_slot(s):
            n = slot_n[s]
            w = n + 8
            off = sum(slot_n[:s]) + 8 * s
            xall, tbc, amtile = load_slot(s, n, off)
            qkt = project(s, n, xall)
            ncha = (n + 127) // 128
            nch = min(2, ncha) if s in cons_off else ncha
            for ic in range(nch):
                i0 = 128 * ic
                il = min(128, n - i0)
                E = epool.tile([128, 8, wmax], F16, tag="E")
                holder["E"] = E
                for half in range(2):
                    psc = pscp.tile([128, 4, 512], F32, tag="psc")
                    for bank in range(4):
                        m = 4 * half + bank
                        br, h = m // 4, m % 4
                        g, sub = h // 2, h % 2
                        rows = slice(64 * sub, 64 * sub + 64)
                        nc.tensor.matmul(
                            psc[0:il, bank, 0:w],
                            qkt[(br, g)][rows, 0, i0 : i0 + il],
                            qkt[(br, g)][rows, 1, 0:w],
                            start=True, stop=True,
                        )
                    nc.scalar.activation(
                        out=E[0:il, 4 * half : 4 * half + 4, 0:w],
                        in_=psc[0:il, :, 0:w],
                        func=AF.Exp,
                    )
                chunk_tail(
                    il, w,
                    amap_c=amtile[0:il, ic, 0:w],
                    tw=tbc[0:il, 0:w],
                    vsrow=amtile[0:il, ic, wmax : wmax + 16],
                    rescols=[(0, il, s * 3 + ic)],
                )

        def do_cons():
            amc = xpool.tile([128, 4, wmax + 16], F16, tag="am", name="amcons")
            nc.sync.dma_start(
                out=amc, in_=amap_d[BPC].rearrange("(c p) w -> p c w", p=128))

            E = epool.tile([128, 8, wmax], F16, tag="E")
            holder["E"] = E
            for half in range(2):
                psc = pscp.tile([128, 4, 512], F32, tag="psc")
                for bank in range(4):
                    m = 4 * half + bank
                    br, h = m // 4, m % 4
                    g, sub = h // 2, h % 2
                    rows = slice(64 * sub, 64 * sub + 64)
                    for s in bigs:
                        n = slot_n[s]
                        w = n + 8
                        cc = n - 256
                        p0 = cons_off[s]
                        nc.tensor.matmul(
                            psc[p0 : p0 + 32, bank, 0:wmax],
                            ones1[0:1, 0:32],
                            S_mrow[s],
                            start=True, stop=True,
                        )
                        nc.tensor.matmul(
                            psc[p0 : p0 + cc, bank, 0:w],
                            S_qkt[s][(br, g)][rows, 0, 256:n],
                            S_qkt[s][(br, g)][rows, 1, 0:w],
                            start=False, stop=True,
                            skip_group_check=True,
                        )
                nc.scalar.activation(
                    out=E[0:ptot, 4 * half : 4 * half + 4, 0:wmax],
                    in_=psc[0:ptot, :, 0:wmax],
                    func=AF.Exp,
                )
            chunk_tail(
                ptot, wmax,
                amap_c=amc[0:ptot, 0, 0:wmax],
                tw=amc[0:ptot, 1, 0:wmax],
                vsrow=amc[0:ptot, 0, wmax : wmax + 16],
                rescols=[
                    (cons_off[s], cons_off[s] + slot_n[s] - 256, s * 3 + 2)
                    for s in bigs
                ],
            )

        order_s = [s for s in range(BPC - 1, -1, -1) if s in cons_off]
        order_s += [s for s in range(BPC - 1, -1, -1) if s not in cons_off]
        done = 0
        for s in order_s:
            do_slot(s)
            done += 1
            if done == len(cons_off) + 1 and bigs:
                do_cons()

        # ---- final: sum over partitions, fold 3 cols/slot ----
        pfin_t = pscp.tile([128, 4, 512], F32, tag="psc")
        pfin = pfin_t[0:1, 0, 0 : BPC * 3]
        nc.tensor.matmul(pfin, ones128, res, start=True, stop=True)
        fin_sb = singles.tile([1, BPC, 3], F32, tag="fin_sb")
        nc.vector.tensor_copy(
            out=fin_sb,
            in_=pfin_t[0:1, 0, 0 : BPC * 3].rearrange("p (s c) -> p s c", c=3),
        )
        lt = singles.tile([1, BPC], F32, tag="lt")
        nc.vector.tensor_add(lt, fin_sb[:, :, 0], fin_sb[:, :, 1])
        nc.vector.tensor_add(lt, lt, fin_sb[:, :, 2])
        nc.sync.dma_start(out=loss_d, in_=lt)

    nc.compile()
    return nc


def _prep(inputs):
    me = np.asarray(inputs["molecule_embedding"], np.float32)
    src_mask = np.asarray(inputs["src_mask"]).astype(bool)
    tgt_mask = np.asarray(inputs["tgt_mask"]).astype(bool)
    src_bond = np.asarray(inputs["src_bond"]).astype(np.int64)
    tgt_bond = np.asarray(inputs["tgt_bond"]).astype(np.int64)

    def f64(k):
        return np.asarray(inputs[k], np.float64)

    # compose (pointwise conv -> in_proj) into one weight; q side gets hd^-.5
    wall = np.zeros((4, DIM, DIM), NP_EDT)
    wbias = np.zeros((4, 128, 2), np.float32)
    for p, (pre, qk_) in enumerate(
        (("inc", "q"), ("inc", "k"), ("dec", "q"), ("dec", "k"))
    ):
        w2, b2 = f64(f"{pre}_w{qk_}"), f64(f"{pre}_b{qk_}")
        w1, b1 = f64(f"{pre}_{qk_}_w"), f64(f"{pre}_{qk_}_b")
        W = w2 @ w1
        bvec = w2 @ b1 + b2
        if qk_ == "q":
            W *= HD ** -0.5
            bvec *= HD ** -0.5
        wall[p] = W.T.astype(NP_EDT)
        wbias[p, :, 0] = bvec[0:128].astype(np.float32)
        wbias[p, :, 1] = bvec[128:256].astype(np.float32)

    kept = [np.nonzero(~src_mask[b])[0] for b in range(B)]
    nk = np.array([len(k) for k in kept])
    order = np.argsort(nk, kind="stable")
    slot_n = []
    for s in range(BPC):
        mx = nk[order[s * NCORES : (s + 1) * NCORES]].max()
        slot_n.append(int(-(-mx // 8) * 8))
    totc = int(sum(slot_n)) + 8 * BPC
    nmax = max(max(slot_n), 64)
    assert nmax <= 504
    wmax = nmax + 8
    bigs, cons_off, ptot = _cons_split(slot_n)

    in_maps = []
    for c in range(NCORES):
        xT = np.zeros((DIM, totc), NP_EDT)
        amap = np.zeros((BPC + 1, 512, wmax + 16), NP_EDT)
        vs = amap[:, 0:384, wmax:]
        tjrow = np.zeros((BPC, wmax), NP_EDT)

        off = 0
        for s in range(BPC):
            n = slot_n[s]
            w = n + 8
            b = int(order[s * NCORES + c])
            kb = kept[b]
            m = len(kb)
            xT[0:DIM, off : off + m] = me[kb, b, :].T.astype(NP_EDT)
            amap[s, 384, m:wmax] = -100.0
            tb = tgt_mask[b, kb].astype(np.float32)
            tjrow[s, 0:m] = tb.astype(NP_EDT)

            # A = C_src - C_tgt * (1-t_i)(1-t_j), compact
            remap = np.full(L, -1, np.int64)
            remap[kb] = np.arange(m)
            A = np.zeros((m, m), np.float32)
            rows = np.repeat(np.arange(m), MB)
            cs = remap[src_bond[b, kb, :]].ravel()
            ok = cs >= 0
            np.add.at(A, (rows[ok], cs[ok]), 1.0)
            Ct = np.zeros((m, m), np.float32)
            ct_ = remap[tgt_bond[b, kb, :]].ravel()
            ok = ct_ >= 0
            np.add.at(Ct, (rows[ok], ct_[ok]), 1.0)
            A -= Ct * np.outer(1.0 - tb, 1.0 - tb)
            Af = A.astype(NP_EDT)

            ncha = (n + 127) // 128
            nch = min(2, ncha) if s in cons_off else ncha
            for ic in range(nch):
                i0 = ic * 128
                ilr = max(0, min(128, m - i0))
                if ilr > 0:
                    amap[s, i0 : i0 + ilr, 0:m] = Af[i0 : i0 + ilr, :]
                    vs[s, i0 : i0 + ilr, 0:4] = 1.0
                    vs[s, i0 : i0 + ilr, 4:8] = -1.0
                    vs[s, i0 : i0 + ilr, 8] = -tb[i0 : i0 + ilr]
                    vs[s, i0 : i0 + ilr, 9] = float(w - m)
                    vs[s, i0 : i0 + ilr, 10] = -float(w - m)
            if s in cons_off:
                p0 = cons_off[s]
                amap[BPC, 128 + p0 : 128 + p0 + (n - 256), 0:m] = tb.astype(
                    NP_EDT
                )[None, :]
                ilr = max(0, m - 256)
                if ilr > 0:
                    amap[BPC, p0 : p0 + ilr, 0:m] = Af[256 : 256 + ilr, :]
                    vs[BPC, p0 : p0 + ilr, 0:4] = 1.0
                    vs[BPC, p0 : p0 + ilr, 4:8] = -1.0
                    vs[BPC, p0 : p0 + ilr, 8] = -tb[256 : 256 + ilr]
            off += w

        in_maps.append(
            {
                "xT": xT,
                "wall": wall,
                "wbias": wbias,
                "amap": amap,
                "tjrow": tjrow,
            }
        )
    return in_maps, tuple(slot_n), nmax, totc, order


def kernel(**inputs) -> np.ndarray:
    in_maps, slot_n, nmax, totc, order = _prep(inputs)
    key = (slot_n, nmax, totc, str(EDT))
    if key not in _CACHE:
        _CACHE[key] = _build_program(list(slot_n), nmax, totc)
    nc = _CACHE[key]
    res = bass_utils.run_bass_kernel_spmd(
        nc,
        in_maps,
        core_ids=list(range(NCORES)),
        trace=False,
    )
    global LAST_RESULTS
    LAST_RESULTS = res
    loss = np.zeros(B, np.float32)
    for c in range(NCORES):
        per_core = res.results[c]["loss"].reshape(BPC)
        for s in range(BPC):
            loss[order[s * NCORES + c]] = per_core[s]
    return loss


LAST_RESULTS = None
